# revision 2
# baseline (speedup 1.0000x reference)
"""Bag-of-words histogram kernel for Trainium2 (8 NeuronCores, data-parallel).

Problem: inputs [1024, 512] int32 token ids in [0, 50257); output [1024, 50256]
f32 per-row token-count histogram with token id 0 dropped.

Design (per core, 128 rows):
  Shifted token u = t - 1 decomposes as u = hi*394 + lo with hi in [-1,128),
  lo in [0,394) (exact integer div via multiply-arith-shift: hi =
  (u*21291)>>23, verified offline for all u in [-1, 50256)). Token t=0 gives
  hi=-1 which matches no hi-bin, so the "drop token 0" semantics falls out of
  the decomposition for free and output bin v = u = hi*394 + lo directly.

  Per row the histogram is hist[hi, lo] = sum_j onehot_hi(u_j)[hi] *
  onehot_lo(u_j)[lo]: a matmul with lhsT = A [K=128 tokens, M=128 hi-bins]
  f16 one-hot, rhs = B [K=128 tokens, N=394 lo-bins] f16 one-hot, accumulated
  over 4 K-chunks of 128 tokens into one PSUM bank [128, 394] f32. Duplicate
  tokens are handled exactly by the accumulation.

  Engine split: DVE builds both one-hots per chunk (tensor_scalar is_equal
  against constant iota rows, f16 so the 4x DVE perf mode applies, with
  per-partition f32 scalars = DMA-transposed token digits). ACT evacuates
  PSUM -> SBUF f32. The output row goes to HBM in 3 pieces per row ([1,394]
  head / [126,394] big / [1,218] tail), the big piece alternating across the
  SP and ACT HWDGE rings.

  Hardware-validated dead ends (all look fine/faster in CoreSim but are
  catastrophically slow on the device):
    - GPSIMD tensor ops: ~2us real launch overhead each (cost model: 107ns).
    - Batched multi-row output DMAs with 3D access patterns, single-partition
      strided-segment DMAs, and 127-partition base-0 DMAs: each such
      dma_start costs ~8us on HW (descriptor-generation slow path) vs ~0.6us
      for the 126-partition 2D shape used here. These were measured at
      ~1.1ms/kernel vs ~0.13-0.26ms for this shape.

  Session-2 findings (HW-measured via engine-isolation dyn-rep variants):
    - This kernel is 100% DVE-bound: a variant with ONLY the 1024 one-hot
      tensor_scalar ops (no matmul/evac/DMA) times identically (~194us best
      case); matmul, evac, and all DMA issue/data fit in the DVE shadow.
    - HW cost of a DVE tensor_scalar with per-partition scalar AP
      (InstTensorScalarPtr): ~122 cycles overhead + FD/4 streaming (4x mode
      IS active; immediate-scalar overhead is ~45cy, so the ptr-scalar load
      penalty is ~77cy). 8 ops/row x 128 rows => ~1500cy/row floor, which
      this kernel already achieves. The two-op form (subtract, is_equal 0)
      costs the same; f16 scalars are rejected (bass requires f32 for
      is_equal).
    - Merging the hi+lo compares into one instruction is provably impossible
      in the (in op0 s1) op1 s2 repertoire: one threshold/target cannot
      serve two independent per-partition match fields (checked xor/sub/and
      x eq/unsigned-lt encodings; the two match residues always clash).
    - ACT offload of the hi one-hot via Abs then Relu(1-x) (stock funcs,
      per-partition bias) is correct but measures ~+550ns/row SLOWER even
      when software-pipelined 2 rows ahead - cross-engine latency lands on
      the PE accumulation chain. Custom PWP activations need compiler
      rebuilds (not NEFF-shippable); custom DVE Specs run at 1 elem/cycle
      (4x slower streaming than stock f16 tensor_scalar).
    - A stash rework (pi-rotated hi->partition map putting head/tail on
      adjacent partitions 126/127, one [2,394] SBUF->SBUF combo DMA per row
      into a staging tile + 2 batched 128-partition DMAs per pass) is
      correct (sim-verified) but a statistical tie with this 3-piece
      scheme - the ring budget already fits under DVE either way.
    - GPSIMD LocalScatter/dma_scatter_add paths lose on launch overhead,
      int16 index ceilings, descriptor-per-token generation, or doubled HBM
      traffic (zero-init + RMW). Perfect-row-split R2=698 (72x698 = exactly
      50256, single DMA/row, no head/tail) inflates DVE area 522->770 per
      token-chunk and loses ~40us. Roofline: output writes 25.7MB/core at
      ~358GB/s => ~72us floor, unreachable while one-hot building is on DVE.
    - SBUF AP encoding note: AP dim 0 must stride in whole-tile-pitch units
      (partitions); [[pitch,1],[R2,2],[1,R2]] addresses two 394-col spans
      within one partition.
"""

import sys

sys.path.insert(0, "/opt/trn_rl_repo")

import numpy as np

N_CORES = 8
B_FULL = 1024
P = 128  # rows per core / partitions
S = 512  # tokens per row
V = 50257
R1 = 128  # hi bins (partition dim of PSUM)
R2 = 394  # lo bins (free dim of PSUM)
DIV_MUL = 21291  # (u*DIV_MUL)>>23 (arith) == u//394 for all u in [-1, 50256)
DIV_SH = 23
NCH = S // P  # 4 K-chunks per row
OUT_COLS = V - 1  # 50256

BIG = 127 * R2  # 50038: v = 0..50037 from partitions 0..126
TAIL = OUT_COLS - BIG  # 218: v = 50038..50255 from partition 127 cols 0..217

_CACHED = {}


def _emit(nc, tc, tile, mybir, tok_dram, out_dram, dyn_reps=None):
    alu = mybir.AluOpType

    with (
        tc.tile_pool(name="const", bufs=1) as const,
        tc.tile_pool(name="prep", bufs=1) as prep,
        tc.tile_pool(name="oh_a", bufs=28) as oh_a,
        tc.tile_pool(name="oh_b", bufs=28) as oh_b,
        tc.tile_pool(name="evac", bufs=10) as evac,
        tc.tile_pool(name="psum", bufs=8, space="PSUM") as psum,
    ):
        # constant iota rows (same in every partition)
        iota_i16 = const.tile([P, R2], mybir.dt.int16)
        nc.gpsimd.iota(iota_i16[:], pattern=[[1, R2]], base=0, channel_multiplier=0)
        iota_lo = const.tile([P, R2], mybir.dt.float16)
        nc.vector.tensor_copy(iota_lo[:], iota_i16[:])
        iota_hi = const.tile([P, R1], mybir.dt.float16)
        nc.vector.tensor_copy(iota_hi[:], iota_i16[:, :R1])

        # load tokens, shift by -1, split digits
        tok = prep.tile([P, S], mybir.dt.int32)
        nc.sync.dma_start(tok[:], tok_dram[:])
        u32 = prep.tile([P, S], mybir.dt.int32)
        nc.vector.tensor_scalar(u32[:], tok[:], 1, None, op0=alu.subtract)
        hprod = prep.tile([P, S], mybir.dt.int32)
        nc.vector.tensor_scalar(hprod[:], u32[:], DIV_MUL, None, op0=alu.mult)
        hi32 = prep.tile([P, S], mybir.dt.int32)
        nc.vector.tensor_scalar(
            hi32[:], hprod[:], DIV_SH, None, op0=alu.arith_shift_right
        )
        him = prep.tile([P, S], mybir.dt.int32)
        nc.vector.tensor_scalar(him[:], hi32[:], R2, None, op0=alu.mult)
        lo32 = prep.tile([P, S], mybir.dt.int32)
        nc.vector.tensor_tensor(lo32[:], u32[:], him[:], op=alu.subtract)

        hi16 = prep.tile([P, S], mybir.dt.int16)
        nc.vector.tensor_copy(hi16[:], hi32[:])
        lo16 = prep.tile([P, S], mybir.dt.int16)
        nc.vector.tensor_copy(lo16[:], lo32[:])

        # transpose each 128-col chunk: digT[p, c*128+b] = dig[b, c*128+p]
        hiT16 = prep.tile([P, S], mybir.dt.int16)
        loT16 = prep.tile([P, S], mybir.dt.int16)
        for c in range(NCH):
            sl = slice(c * P, (c + 1) * P)
            nc.sync.dma_start_transpose(hiT16[:, sl], hi16[:, sl])
            nc.sync.dma_start_transpose(loT16[:, sl], lo16[:, sl])

        # f32 per-partition scalar sources
        hiT = prep.tile([P, S], mybir.dt.float32)
        nc.vector.tensor_copy(hiT[:], hiT16[:])
        loT = prep.tile([P, S], mybir.dt.float32)
        nc.vector.tensor_copy(loT[:], loT16[:])

        def row_body(b):
            ps = psum.tile([P, R2], mybir.dt.float32)
            for c in range(NCH):
                col = c * P + b
                a_t = oh_a.tile([P, R1], mybir.dt.float16)
                b_t = oh_b.tile([P, R2], mybir.dt.float16)
                nc.vector.tensor_scalar(
                    a_t[:], iota_hi[:], hiT[:, col : col + 1], None, op0=alu.is_equal
                )
                nc.vector.tensor_scalar(
                    b_t[:], iota_lo[:], loT[:, col : col + 1], None, op0=alu.is_equal
                )
                nc.tensor.matmul(
                    ps[:], a_t[:], b_t[:], start=(c == 0), stop=(c == NCH - 1)
                )
            # DMA cannot read PSUM; evacuate via otherwise-idle ScalarE
            ev = evac.tile([P, R2], mybir.dt.float32)
            nc.scalar.copy(ev[:], ps[:])
            # v = partition*394 + lo directly; 3 pieces: [1,394] head /
            # [126,394] big / [1,218] tail, alternating the big piece across
            # both HWDGE rings. NOTE: merging the head into a 127-partition
            # base-0 big piece looks free in CoreSim but costs ~8us per DMA
            # on real hardware (odd-partition-count descriptor slow path) —
            # keep the 126-partition shape.
            big, small = (nc.sync, nc.scalar) if b % 2 == 0 else (nc.scalar, nc.sync)
            small.dma_start(out_dram[b : b + 1, 0:R2], ev[0:1, :])
            big.dma_start(out_dram[b, R2:BIG], ev[1:127, :])
            small.dma_start(out_dram[b : b + 1, BIG:OUT_COLS], ev[127:128, 0:TAIL])

        if dyn_reps is None:
            for b in range(P):
                row_body(b)
        else:
            with tc.For_i(0, dyn_reps, 1):
                for b in range(P):
                    row_body(b)


def _build_program():
    import concourse.tile as tile
    from concourse import bacc, mybir

    nc = bacc.Bacc(
        "TRN2",
        target_bir_lowering=False,
        debug=False,
        enable_asserts=False,
        num_devices=N_CORES,
    )
    tok_dram = nc.dram_tensor("inputs", [P, S], mybir.dt.int32, kind="ExternalInput").ap()
    out_dram = nc.dram_tensor(
        "out", [P, OUT_COLS], mybir.dt.float32, kind="ExternalOutput"
    ).ap()

    with tile.TileContext(nc) as tc:
        _emit(nc, tc, tile, mybir, tok_dram, out_dram)

    nc.compile()
    return nc


def _build_program_dyn():
    """Variant with a runtime repeat loop around the row loop, for HW timing.

    Trip count comes from the extra [1,1] uint32 input "reps" — same NEFF for
    any R, so wall-time slope over R isolates device execution time.
    """
    import concourse.tile as tile
    from concourse import bacc, mybir

    nc = bacc.Bacc(
        "TRN2",
        target_bir_lowering=False,
        debug=False,
        enable_asserts=False,
        num_devices=N_CORES,
    )
    tok_dram = nc.dram_tensor("inputs", [P, S], mybir.dt.int32, kind="ExternalInput").ap()
    reps_dram = nc.dram_tensor("reps", [1, 1], mybir.dt.uint32, kind="ExternalInput").ap()
    out_dram = nc.dram_tensor(
        "out", [P, OUT_COLS], mybir.dt.float32, kind="ExternalOutput"
    ).ap()

    with tile.TileContext(nc) as tc:
        with tc.tile_pool(name="repsld", bufs=1) as repsld:
            reps_sb = repsld.tile([1, 1], mybir.dt.uint32)
            nc.sync.dma_start(reps_sb[:], reps_dram[:])

            from concourse.bass_primitives_rust import RegisterHandles
            from concourse.expressions import make_scalar_value

            regs = []
            for eng in (nc.sync, nc.vector, nc.scalar, nc.tensor, nc.gpsimd):
                tmp = eng.alloc_register(f"reps_{eng.engine.value}")
                eng.reg_load(tmp, reps_sb[0:1, 0:1])
                regs.append(tmp)
            rv = make_scalar_value(RegisterHandles(regs), min_val=0, max_val=1 << 20)

            _emit(nc, tc, tile, mybir, tok_dram, out_dram, dyn_reps=rv)

    nc.compile()
    return nc


def _get_program():
    if "nc" not in _CACHED:
        _CACHED["nc"] = _build_program()
    return _CACHED["nc"]


def kernel(inputs: np.ndarray, _trace: bool = False, _tmpdir: str | None = None):
    from concourse.bass_utils import run_bass_kernel_spmd

    nc = _get_program()
    inputs = np.ascontiguousarray(np.asarray(inputs, dtype=np.int32))
    assert inputs.shape == (B_FULL, S), inputs.shape
    in_maps = [{"inputs": inputs[k * P : (k + 1) * P]} for k in range(N_CORES)]
    res = run_bass_kernel_spmd(
        nc,
        in_maps,
        core_ids=list(range(N_CORES)),
        trace=_trace,
        tmpdir=_tmpdir,
    )
    out = np.concatenate([r["out"] for r in res.results], axis=0)
    if _trace:
        _CACHED["last_results"] = res
    return out



# revision 3
# speedup vs baseline: 1.2406x; 1.2406x over previous
"""Bag-of-words histogram kernel for Trainium2 (8 NeuronCores, data-parallel).

Problem: inputs [1024, 512] int32 token ids in [0, 50257); output [1024, 50256]
f32 per-row token-count histogram with token id 0 dropped.

Design (per core, 128 rows):
  Shifted token u = t - 1 decomposes as u = hi*394 + lo with hi in [-1,128),
  lo in [0,394) (exact integer div via multiply-arith-shift: hi =
  (u*21291)>>23, verified offline for all u in [-1, 50256)). Token t=0 gives
  hi=-1 which matches no hi-bin, so the "drop token 0" semantics falls out of
  the decomposition for free and output bin v = u = hi*394 + lo directly.

  Per row the histogram is hist[hi, lo] = sum_j onehot_hi(u_j)[hi] *
  onehot_lo(u_j)[lo]: a matmul with lhsT = A [K=128 tokens, M=128 hi-bins]
  f16 one-hot, rhs = B [K=128 tokens, N=394 lo-bins] f16 one-hot, accumulated
  over 4 K-chunks of 128 tokens into one PSUM bank [128, 394] f32. Duplicate
  tokens are handled exactly by the accumulation.

  Engine split: DVE builds both one-hots per chunk (tensor_scalar is_equal
  against constant iota rows, f16 so the 4x DVE perf mode applies, with
  per-partition f32 scalars = DMA-transposed token digits). ACT evacuates
  PSUM -> SBUF f32. The output row goes to HBM in 3 pieces per row ([1,394]
  head / [126,394] big / [1,218] tail), the big piece alternating across the
  SP and ACT HWDGE rings.

  Hardware-validated dead ends (all look fine/faster in CoreSim but are
  catastrophically slow on the device):
    - GPSIMD tensor ops: ~2us real launch overhead each (cost model: 107ns).
    - Batched multi-row output DMAs with 3D access patterns, single-partition
      strided-segment DMAs, and 127-partition base-0 DMAs: each such
      dma_start costs ~8us on HW (descriptor-generation slow path) vs ~0.6us
      for the 126-partition 2D shape used here. These were measured at
      ~1.1ms/kernel vs ~0.13-0.26ms for this shape.

  Session-2 findings (HW-measured via engine-isolation dyn-rep variants):
    - This kernel is 100% DVE-bound: a variant with ONLY the 1024 one-hot
      tensor_scalar ops (no matmul/evac/DMA) times identically (~194us best
      case); matmul, evac, and all DMA issue/data fit in the DVE shadow.
    - HW cost of a DVE tensor_scalar with per-partition scalar AP
      (InstTensorScalarPtr): ~122 cycles overhead + FD/4 streaming (4x mode
      IS active; immediate-scalar overhead is ~45cy, so the ptr-scalar load
      penalty is ~77cy). 8 ops/row x 128 rows => ~1500cy/row floor, which
      this kernel already achieves. The two-op form (subtract, is_equal 0)
      costs the same; f16 scalars are rejected (bass requires f32 for
      is_equal).
    - Merging the hi+lo compares into one instruction is provably impossible
      in the (in op0 s1) op1 s2 repertoire: one threshold/target cannot
      serve two independent per-partition match fields (checked xor/sub/and
      x eq/unsigned-lt encodings; the two match residues always clash).
    - ACT offload of the hi one-hot via Abs then Relu(1-x) (stock funcs,
      per-partition bias) is correct but measures +25-40% SLOWER at both
      2-row and 6-row software-pipeline depths - bias-AP activations are
      far costlier on HW than the 224+FD/Accel model. Custom PWP activations need compiler
      rebuilds (not NEFF-shippable); custom DVE Specs run at 1 elem/cycle
      (4x slower streaming than stock f16 tensor_scalar).
    - A stash rework (pi-rotated hi->partition map putting head/tail on
      adjacent partitions 126/127, one [2,394] SBUF->SBUF combo DMA per row
      into a staging tile + 2 batched 128-partition DMAs per pass) is
      correct (sim-verified) but a statistical tie with this 3-piece
      scheme - the ring budget already fits under DVE either way.
    - GPSIMD LocalScatter/dma_scatter_add paths lose on launch overhead,
      int16 index ceilings, descriptor-per-token generation, or doubled HBM
      traffic (zero-init + RMW). Perfect-row-split R2=698 (72x698 = exactly
      50256, single DMA/row, no head/tail) inflates DVE area 522->770 per
      token-chunk and loses ~40us. Roofline: output writes 25.7MB/core at
      ~358GB/s => ~72us floor, unreachable while one-hot building is on DVE.
    - SBUF AP encoding note: AP dim 0 must stride in whole-tile-pitch units
      (partitions); [[pitch,1],[R2,2],[1,R2]] addresses two 394-col spans
      within one partition.
"""

import sys

sys.path.insert(0, "/opt/trn_rl_repo")

import numpy as np

N_CORES = 8
B_FULL = 1024
P = 128  # rows per core / partitions
S = 512  # tokens per row
V = 50257
R1 = 128  # hi bins (partition dim of PSUM)
R2 = 394  # lo bins (free dim of PSUM)
DIV_MUL = 21291  # (u*DIV_MUL)>>23 (arith) == u//394 for all u in [-1, 50256)
DIV_SH = 23
NCH = S // P  # 4 K-chunks per row
OUT_COLS = V - 1  # 50256

BIG = 127 * R2  # 50038: v = 0..50037 from partitions 0..126
TAIL = OUT_COLS - BIG  # 218: v = 50038..50255 from partition 127 cols 0..217

_CACHED = {}


def _emit(nc, tc, tile, mybir, tok_dram, out_dram, dyn_reps=None):
    alu = mybir.AluOpType

    with (
        tc.tile_pool(name="const", bufs=1) as const,
        tc.tile_pool(name="prep", bufs=1) as prep,
        tc.tile_pool(name="oh_a", bufs=28) as oh_a,
        tc.tile_pool(name="oh_b", bufs=28) as oh_b,
        tc.tile_pool(name="evac", bufs=10) as evac,
        tc.tile_pool(name="psum", bufs=8, space="PSUM") as psum,
    ):
        # constant iota rows (same in every partition)
        iota_i16 = const.tile([P, R2], mybir.dt.int16)
        nc.gpsimd.iota(iota_i16[:], pattern=[[1, R2]], base=0, channel_multiplier=0)
        iota_lo = const.tile([P, R2], mybir.dt.float16)
        nc.vector.tensor_copy(iota_lo[:], iota_i16[:])
        iota_hi = const.tile([P, R1], mybir.dt.float16)
        nc.vector.tensor_copy(iota_hi[:], iota_i16[:, :R1])

        # load tokens, shift by -1, split digits
        tok = prep.tile([P, S], mybir.dt.int32)
        nc.sync.dma_start(tok[:], tok_dram[:])
        u32 = prep.tile([P, S], mybir.dt.int32)
        nc.vector.tensor_scalar(u32[:], tok[:], 1, None, op0=alu.subtract)
        hprod = prep.tile([P, S], mybir.dt.int32)
        nc.vector.tensor_scalar(hprod[:], u32[:], DIV_MUL, None, op0=alu.mult)
        hi32 = prep.tile([P, S], mybir.dt.int32)
        nc.vector.tensor_scalar(
            hi32[:], hprod[:], DIV_SH, None, op0=alu.arith_shift_right
        )
        him = prep.tile([P, S], mybir.dt.int32)
        nc.vector.tensor_scalar(him[:], hi32[:], R2, None, op0=alu.mult)
        lo32 = prep.tile([P, S], mybir.dt.int32)
        nc.vector.tensor_tensor(lo32[:], u32[:], him[:], op=alu.subtract)

        hi16 = prep.tile([P, S], mybir.dt.int16)
        nc.vector.tensor_copy(hi16[:], hi32[:])
        lo16 = prep.tile([P, S], mybir.dt.int16)
        nc.vector.tensor_copy(lo16[:], lo32[:])

        # transpose each 128-col chunk: digT[p, c*128+b] = dig[b, c*128+p]
        hiT16 = prep.tile([P, S], mybir.dt.int16)
        loT16 = prep.tile([P, S], mybir.dt.int16)
        for c in range(NCH):
            sl = slice(c * P, (c + 1) * P)
            nc.sync.dma_start_transpose(hiT16[:, sl], hi16[:, sl])
            nc.sync.dma_start_transpose(loT16[:, sl], lo16[:, sl])

        # f32 per-partition scalar sources
        hiT = prep.tile([P, S], mybir.dt.float32)
        nc.vector.tensor_copy(hiT[:], hiT16[:])
        loT = prep.tile([P, S], mybir.dt.float32)
        nc.vector.tensor_copy(loT[:], loT16[:])

        def row_body(b):
            ps = psum.tile([P, R2], mybir.dt.float32)
            for c in range(NCH):
                col = c * P + b
                a_t = oh_a.tile([P, R1], mybir.dt.float16)
                b_t = oh_b.tile([P, R2], mybir.dt.float16)
                nc.vector.tensor_scalar(
                    a_t[:], iota_hi[:], hiT[:, col : col + 1], None, op0=alu.is_equal
                )
                nc.vector.tensor_scalar(
                    b_t[:], iota_lo[:], loT[:, col : col + 1], None, op0=alu.is_equal
                )
                nc.tensor.matmul(
                    ps[:], a_t[:], b_t[:], start=(c == 0), stop=(c == NCH - 1)
                )
            # DMA cannot read PSUM; evacuate via otherwise-idle ScalarE
            ev = evac.tile([P, R2], mybir.dt.float32)
            nc.scalar.copy(ev[:], ps[:])
            # v = partition*394 + lo directly; 3 pieces: [1,394] head /
            # [126,394] big / [1,218] tail, alternating the big piece across
            # both HWDGE rings. NOTE: merging the head into a 127-partition
            # base-0 big piece looks free in CoreSim but costs ~8us per DMA
            # on real hardware (odd-partition-count descriptor slow path) —
            # keep the 126-partition shape.
            big, small = (nc.sync, nc.scalar) if b % 2 == 0 else (nc.scalar, nc.sync)
            small.dma_start(out_dram[b : b + 1, 0:R2], ev[0:1, :])
            big.dma_start(out_dram[b, R2:BIG], ev[1:127, :])
            small.dma_start(out_dram[b : b + 1, BIG:OUT_COLS], ev[127:128, 0:TAIL])

        if dyn_reps is None:
            for b in range(P):
                row_body(b)
        else:
            with tc.For_i(0, dyn_reps, 1):
                for b in range(P):
                    row_body(b)


def _build_program():
    import concourse.tile as tile
    from concourse import bacc, mybir

    nc = bacc.Bacc(
        "TRN2",
        target_bir_lowering=False,
        debug=False,
        enable_asserts=False,
        num_devices=N_CORES,
    )
    tok_dram = nc.dram_tensor("inputs", [P, S], mybir.dt.int32, kind="ExternalInput").ap()
    out_dram = nc.dram_tensor(
        "out", [P, OUT_COLS], mybir.dt.float32, kind="ExternalOutput"
    ).ap()

    with tile.TileContext(nc) as tc:
        _emit(nc, tc, tile, mybir, tok_dram, out_dram)

    nc.compile()
    return nc


def _build_program_dyn():
    """Variant with a runtime repeat loop around the row loop, for HW timing.

    Trip count comes from the extra [1,1] uint32 input "reps" — same NEFF for
    any R, so wall-time slope over R isolates device execution time.
    """
    import concourse.tile as tile
    from concourse import bacc, mybir

    nc = bacc.Bacc(
        "TRN2",
        target_bir_lowering=False,
        debug=False,
        enable_asserts=False,
        num_devices=N_CORES,
    )
    tok_dram = nc.dram_tensor("inputs", [P, S], mybir.dt.int32, kind="ExternalInput").ap()
    reps_dram = nc.dram_tensor("reps", [1, 1], mybir.dt.uint32, kind="ExternalInput").ap()
    out_dram = nc.dram_tensor(
        "out", [P, OUT_COLS], mybir.dt.float32, kind="ExternalOutput"
    ).ap()

    with tile.TileContext(nc) as tc:
        with tc.tile_pool(name="repsld", bufs=1) as repsld:
            reps_sb = repsld.tile([1, 1], mybir.dt.uint32)
            nc.sync.dma_start(reps_sb[:], reps_dram[:])

            from concourse.bass_primitives_rust import RegisterHandles
            from concourse.expressions import make_scalar_value

            regs = []
            for eng in (nc.sync, nc.vector, nc.scalar, nc.tensor, nc.gpsimd):
                tmp = eng.alloc_register(f"reps_{eng.engine.value}")
                eng.reg_load(tmp, reps_sb[0:1, 0:1])
                regs.append(tmp)
            rv = make_scalar_value(RegisterHandles(regs), min_val=0, max_val=1 << 20)

            _emit(nc, tc, tile, mybir, tok_dram, out_dram, dyn_reps=rv)

    nc.compile()
    return nc


def _get_program():
    if "nc" not in _CACHED:
        _CACHED["nc"] = _build_program()
    return _CACHED["nc"]


def kernel(inputs: np.ndarray, _trace: bool = False, _tmpdir: str | None = None):
    from concourse.bass_utils import run_bass_kernel_spmd

    nc = _get_program()
    inputs = np.ascontiguousarray(np.asarray(inputs, dtype=np.int32))
    assert inputs.shape == (B_FULL, S), inputs.shape
    in_maps = [{"inputs": inputs[k * P : (k + 1) * P]} for k in range(N_CORES)]
    res = run_bass_kernel_spmd(
        nc,
        in_maps,
        core_ids=list(range(N_CORES)),
        trace=_trace,
        tmpdir=_tmpdir,
    )
    out = np.concatenate([r["out"] for r in res.results], axis=0)
    if _trace:
        _CACHED["last_results"] = res
    return out



# revision 4
# speedup vs baseline: 1.2435x; 1.0023x over previous
"""Bag-of-words histogram kernel for Trainium2 (8 NeuronCores, data-parallel).

Problem: inputs [1024, 512] int32 token ids in [0, 50257); output [1024, 50256]
f32 per-row token-count histogram with token id 0 dropped.

Design (per core, 128 rows):
  Shifted token u = t - 1 decomposes as u = hi*394 + lo with hi in [-1,128),
  lo in [0,394) (exact integer div via multiply-arith-shift: hi =
  (u*21291)>>23, verified offline for all u in [-1, 50256)). Token t=0 gives
  hi=-1 which matches no hi-bin, so the "drop token 0" semantics falls out of
  the decomposition for free and output bin v = u = hi*394 + lo directly.

  Per row the histogram is hist[hi, lo] = sum_j onehot_hi(u_j)[hi] *
  onehot_lo(u_j)[lo]: a matmul with lhsT = A [K=128 tokens, M=128 hi-bins]
  f16 one-hot, rhs = B [K=128 tokens, N=394 lo-bins] f16 one-hot, accumulated
  over 4 K-chunks of 128 tokens into one PSUM bank [128, 394] f32. Duplicate
  tokens are handled exactly by the accumulation.

  Engine split: DVE builds both one-hots per chunk (tensor_scalar is_equal
  against constant iota rows, f16 so the 4x DVE perf mode applies, with
  per-partition f32 scalars = DMA-transposed token digits). ACT evacuates
  PSUM -> SBUF f32. The output row goes to HBM in 3 pieces per row ([1,394]
  head / [126,394] big / [1,218] tail), the big piece alternating across the
  SP and ACT HWDGE rings.

  Hardware-validated dead ends (all look fine/faster in CoreSim but are
  catastrophically slow on the device):
    - GPSIMD tensor ops: ~2us real launch overhead each (cost model: 107ns).
    - Batched multi-row output DMAs with 3D access patterns, single-partition
      strided-segment DMAs, and 127-partition base-0 DMAs: each such
      dma_start costs ~8us on HW (descriptor-generation slow path) vs ~0.6us
      for the 126-partition 2D shape used here. These were measured at
      ~1.1ms/kernel vs ~0.13-0.26ms for this shape.

  Session-2 findings (HW-measured via engine-isolation dyn-rep variants):
    - This kernel is 100% DVE-bound: a variant with ONLY the 1024 one-hot
      tensor_scalar ops (no matmul/evac/DMA) times identically (~194us best
      case); matmul, evac, and all DMA issue/data fit in the DVE shadow.
    - HW cost of a DVE tensor_scalar with per-partition scalar AP
      (InstTensorScalarPtr): ~122 cycles overhead + FD/4 streaming (4x mode
      IS active; immediate-scalar overhead is ~45cy, so the ptr-scalar load
      penalty is ~77cy). 8 ops/row x 128 rows => ~1500cy/row floor, which
      this kernel already achieves. The two-op form (subtract, is_equal 0)
      costs the same; f16 scalars are rejected (bass requires f32 for
      is_equal).
    - Merging the hi+lo compares into one instruction is provably impossible
      in the (in op0 s1) op1 s2 repertoire: one threshold/target cannot
      serve two independent per-partition match fields (checked xor/sub/and
      x eq/unsigned-lt encodings; the two match residues always clash).
    - ACT offload of the hi one-hot via Abs then Relu(1-x) (stock funcs,
      per-partition bias) is correct but measures +25-40% SLOWER at both
      2-row and 6-row software-pipeline depths - bias-AP activations are
      far costlier on HW than the 224+FD/Accel model. Custom PWP activations need compiler
      rebuilds (not NEFF-shippable); custom DVE Specs run at 1 elem/cycle
      (4x slower streaming than stock f16 tensor_scalar).
    - A stash rework (pi-rotated hi->partition map putting head/tail on
      adjacent partitions 126/127, one [2,394] SBUF->SBUF combo DMA per row
      into a staging tile + 2 batched 128-partition DMAs per pass) is
      correct (sim-verified) but a statistical tie with this 3-piece
      scheme - the ring budget already fits under DVE either way.
    - GPSIMD LocalScatter/dma_scatter_add paths lose on launch overhead,
      int16 index ceilings, descriptor-per-token generation, or doubled HBM
      traffic (zero-init + RMW). Perfect-row-split R2=698 (72x698 = exactly
      50256, single DMA/row, no head/tail) inflates DVE area 522->770 per
      token-chunk and loses ~40us. Roofline: output writes 25.7MB/core at
      ~358GB/s => ~72us floor, unreachable while one-hot building is on DVE.
    - SBUF AP encoding note: AP dim 0 must stride in whole-tile-pitch units
      (partitions); [[pitch,1],[R2,2],[1,R2]] addresses two 394-col spans
      within one partition.
    - Correctness hardening: the (u*21291)>>23 == u//394 identity is
      exhaustively exact over u in [-1, 50256) with no int32 overflow
      (max product 1.07e9), and the kernel is bit-exact on adversarial
      inputs (all-padding rows, 512x duplicate tokens, dense boundary-token
      mixes: 0/1/393/394/50255/50256).
"""

import sys

sys.path.insert(0, "/opt/trn_rl_repo")

import numpy as np

N_CORES = 8
B_FULL = 1024
P = 128  # rows per core / partitions
S = 512  # tokens per row
V = 50257
R1 = 128  # hi bins (partition dim of PSUM)
R2 = 394  # lo bins (free dim of PSUM)
DIV_MUL = 21291  # (u*DIV_MUL)>>23 (arith) == u//394 for all u in [-1, 50256)
DIV_SH = 23
NCH = S // P  # 4 K-chunks per row
OUT_COLS = V - 1  # 50256

BIG = 127 * R2  # 50038: v = 0..50037 from partitions 0..126
TAIL = OUT_COLS - BIG  # 218: v = 50038..50255 from partition 127 cols 0..217

_CACHED = {}


def _emit(nc, tc, tile, mybir, tok_dram, out_dram, dyn_reps=None):
    alu = mybir.AluOpType

    with (
        tc.tile_pool(name="const", bufs=1) as const,
        tc.tile_pool(name="prep", bufs=1) as prep,
        tc.tile_pool(name="oh_a", bufs=28) as oh_a,
        tc.tile_pool(name="oh_b", bufs=28) as oh_b,
        tc.tile_pool(name="evac", bufs=10) as evac,
        tc.tile_pool(name="psum", bufs=8, space="PSUM") as psum,
    ):
        # constant iota rows (same in every partition)
        iota_i16 = const.tile([P, R2], mybir.dt.int16)
        nc.gpsimd.iota(iota_i16[:], pattern=[[1, R2]], base=0, channel_multiplier=0)
        iota_lo = const.tile([P, R2], mybir.dt.float16)
        nc.vector.tensor_copy(iota_lo[:], iota_i16[:])
        iota_hi = const.tile([P, R1], mybir.dt.float16)
        nc.vector.tensor_copy(iota_hi[:], iota_i16[:, :R1])

        # load tokens, shift by -1, split digits
        tok = prep.tile([P, S], mybir.dt.int32)
        nc.sync.dma_start(tok[:], tok_dram[:])
        u32 = prep.tile([P, S], mybir.dt.int32)
        nc.vector.tensor_scalar(u32[:], tok[:], 1, None, op0=alu.subtract)
        hprod = prep.tile([P, S], mybir.dt.int32)
        nc.vector.tensor_scalar(hprod[:], u32[:], DIV_MUL, None, op0=alu.mult)
        hi32 = prep.tile([P, S], mybir.dt.int32)
        nc.vector.tensor_scalar(
            hi32[:], hprod[:], DIV_SH, None, op0=alu.arith_shift_right
        )
        him = prep.tile([P, S], mybir.dt.int32)
        nc.vector.tensor_scalar(him[:], hi32[:], R2, None, op0=alu.mult)
        lo32 = prep.tile([P, S], mybir.dt.int32)
        nc.vector.tensor_tensor(lo32[:], u32[:], him[:], op=alu.subtract)

        hi16 = prep.tile([P, S], mybir.dt.int16)
        nc.vector.tensor_copy(hi16[:], hi32[:])
        lo16 = prep.tile([P, S], mybir.dt.int16)
        nc.vector.tensor_copy(lo16[:], lo32[:])

        # transpose each 128-col chunk: digT[p, c*128+b] = dig[b, c*128+p]
        hiT16 = prep.tile([P, S], mybir.dt.int16)
        loT16 = prep.tile([P, S], mybir.dt.int16)
        for c in range(NCH):
            sl = slice(c * P, (c + 1) * P)
            nc.sync.dma_start_transpose(hiT16[:, sl], hi16[:, sl])
            nc.sync.dma_start_transpose(loT16[:, sl], lo16[:, sl])

        # f32 per-partition scalar sources
        hiT = prep.tile([P, S], mybir.dt.float32)
        nc.vector.tensor_copy(hiT[:], hiT16[:])
        loT = prep.tile([P, S], mybir.dt.float32)
        nc.vector.tensor_copy(loT[:], loT16[:])

        def row_body(b):
            ps = psum.tile([P, R2], mybir.dt.float32)
            for c in range(NCH):
                col = c * P + b
                a_t = oh_a.tile([P, R1], mybir.dt.float16)
                b_t = oh_b.tile([P, R2], mybir.dt.float16)
                nc.vector.tensor_scalar(
                    a_t[:], iota_hi[:], hiT[:, col : col + 1], None, op0=alu.is_equal
                )
                nc.vector.tensor_scalar(
                    b_t[:], iota_lo[:], loT[:, col : col + 1], None, op0=alu.is_equal
                )
                nc.tensor.matmul(
                    ps[:], a_t[:], b_t[:], start=(c == 0), stop=(c == NCH - 1)
                )
            # DMA cannot read PSUM; evacuate via otherwise-idle ScalarE
            ev = evac.tile([P, R2], mybir.dt.float32)
            nc.scalar.copy(ev[:], ps[:])
            # v = partition*394 + lo directly; 3 pieces: [1,394] head /
            # [126,394] big / [1,218] tail, alternating the big piece across
            # both HWDGE rings. NOTE: merging the head into a 127-partition
            # base-0 big piece looks free in CoreSim but costs ~8us per DMA
            # on real hardware (odd-partition-count descriptor slow path) —
            # keep the 126-partition shape.
            big, small = (nc.sync, nc.scalar) if b % 2 == 0 else (nc.scalar, nc.sync)
            small.dma_start(out_dram[b : b + 1, 0:R2], ev[0:1, :])
            big.dma_start(out_dram[b, R2:BIG], ev[1:127, :])
            small.dma_start(out_dram[b : b + 1, BIG:OUT_COLS], ev[127:128, 0:TAIL])

        if dyn_reps is None:
            for b in range(P):
                row_body(b)
        else:
            with tc.For_i(0, dyn_reps, 1):
                for b in range(P):
                    row_body(b)


def _build_program():
    import concourse.tile as tile
    from concourse import bacc, mybir

    nc = bacc.Bacc(
        "TRN2",
        target_bir_lowering=False,
        debug=False,
        enable_asserts=False,
        num_devices=N_CORES,
    )
    tok_dram = nc.dram_tensor("inputs", [P, S], mybir.dt.int32, kind="ExternalInput").ap()
    out_dram = nc.dram_tensor(
        "out", [P, OUT_COLS], mybir.dt.float32, kind="ExternalOutput"
    ).ap()

    with tile.TileContext(nc) as tc:
        _emit(nc, tc, tile, mybir, tok_dram, out_dram)

    nc.compile()
    return nc


def _build_program_dyn():
    """Variant with a runtime repeat loop around the row loop, for HW timing.

    Trip count comes from the extra [1,1] uint32 input "reps" — same NEFF for
    any R, so wall-time slope over R isolates device execution time.
    """
    import concourse.tile as tile
    from concourse import bacc, mybir

    nc = bacc.Bacc(
        "TRN2",
        target_bir_lowering=False,
        debug=False,
        enable_asserts=False,
        num_devices=N_CORES,
    )
    tok_dram = nc.dram_tensor("inputs", [P, S], mybir.dt.int32, kind="ExternalInput").ap()
    reps_dram = nc.dram_tensor("reps", [1, 1], mybir.dt.uint32, kind="ExternalInput").ap()
    out_dram = nc.dram_tensor(
        "out", [P, OUT_COLS], mybir.dt.float32, kind="ExternalOutput"
    ).ap()

    with tile.TileContext(nc) as tc:
        with tc.tile_pool(name="repsld", bufs=1) as repsld:
            reps_sb = repsld.tile([1, 1], mybir.dt.uint32)
            nc.sync.dma_start(reps_sb[:], reps_dram[:])

            from concourse.bass_primitives_rust import RegisterHandles
            from concourse.expressions import make_scalar_value

            regs = []
            for eng in (nc.sync, nc.vector, nc.scalar, nc.tensor, nc.gpsimd):
                tmp = eng.alloc_register(f"reps_{eng.engine.value}")
                eng.reg_load(tmp, reps_sb[0:1, 0:1])
                regs.append(tmp)
            rv = make_scalar_value(RegisterHandles(regs), min_val=0, max_val=1 << 20)

            _emit(nc, tc, tile, mybir, tok_dram, out_dram, dyn_reps=rv)

    nc.compile()
    return nc


def _get_program():
    if "nc" not in _CACHED:
        _CACHED["nc"] = _build_program()
    return _CACHED["nc"]


def kernel(inputs: np.ndarray, _trace: bool = False, _tmpdir: str | None = None):
    from concourse.bass_utils import run_bass_kernel_spmd

    nc = _get_program()
    inputs = np.ascontiguousarray(np.asarray(inputs, dtype=np.int32))
    assert inputs.shape == (B_FULL, S), inputs.shape
    in_maps = [{"inputs": inputs[k * P : (k + 1) * P]} for k in range(N_CORES)]
    res = run_bass_kernel_spmd(
        nc,
        in_maps,
        core_ids=list(range(N_CORES)),
        trace=_trace,
        tmpdir=_tmpdir,
    )
    out = np.concatenate([r["out"] for r in res.results], axis=0)
    if _trace:
        _CACHED["last_results"] = res
    return out

